# revision 1
# baseline (speedup 1.0000x reference)
"""Trainium2 Bass kernel for a dense-transformer attention block.

Reference semantics (T=2048, D=2048, 16 heads, d_h=128):
    h = RMSNorm(x) * ln_w
    q,k,v = h @ W{q,k,v}.T  -> (n_h, T, d_h);  RoPE(q, k)
    att = softmax(causal(q k^T / sqrt(d_h))) @ v
    out = x + att @ Wo.T          (attention_mask is all-ones per spec)

Distribution: head-parallel over 8 cores (2 heads/core).  Each core:
  phase 1  QKV projections for its heads (bf16 matmuls, contract over d_model);
           RMSNorm folded in: row scales r[t] enter via r-scaled RoPE tables
           (q,k) and per-row scaling (v); ln_w is folded into the weights.
           rotate_half runs on the PE as a constant permutation matmul.
  phase 2  per-head causal attention with scores computed TRANSPOSED
           (S^T[j,i]) so no transposes are needed anywhere; softmax row-sums
           accumulate on the PE via a ones-vector matmul; exp on ScalarE.
  phase 3  per-head AllGather of att^T rows (overlaps the other head's work)
  phase 4  output projection column-shard, weight-stationary:
           out^T[:, cols_c] rows = sum_k WoT-chunk.T @ attT-chunk  + residual
Host assembles out = concat(out_colsT.T, axis=1).
"""

import math

import numpy as np

EPS = 1e-5
NEG = -1.0e30

CFG_FULL = dict(T=2048, D=2048, n_cores=8, heads_per_core=2)


# --------------------------------------------------------------------------
# device program
# --------------------------------------------------------------------------
def build_nc(T, D, n_cores, heads_per_core):
    import concourse.mybir as mybir
    import concourse.tile as tile
    from concourse import bacc

    DH = 128                      # head dim (hard-wired into layout)
    P = 128                       # partitions
    NH = heads_per_core
    DL = NH * DH                  # local width (q/k/v columns per core)
    KC = D // P                   # k-chunks over d_model
    TB = T // 512                 # 512-wide t blocks
    NIB = T // 512                # 512-wide i blocks
    NTS = T // P                  # 128-wide t subtiles
    NPAIR = max(1, NIB // 2)      # t-block pairs (AG / phase-4 granularity)
    f32 = mybir.dt.float32
    bf16 = mybir.dt.bfloat16
    i32 = mybir.dt.int32

    nc = bacc.Bacc("TRN2", target_bir_lowering=False, debug=False,
                   num_devices=n_cores)

    # ---- I/O ----
    xT = nc.dram_tensor("xT", [D, T], bf16, kind="ExternalInput").ap()
    xct_in = nc.dram_tensor("x_colsT", [DL, T], f32, kind="ExternalInput").ap()
    # weight tensors arrive host-pretiled in SBUF layout [P, KC*DL]
    wq_t = nc.dram_tensor("wq_t", [P, KC * DL], bf16, kind="ExternalInput").ap()
    wk_t = nc.dram_tensor("wk_t", [P, KC * DL], bf16, kind="ExternalInput").ap()
    wv_t = nc.dram_tensor("wv_t", [P, KC * DL], bf16, kind="ExternalInput").ap()
    # wo_t additionally row-permuted on host to the AllGather chunk order
    wo_t = nc.dram_tensor("wo_t", [P, KC * DL], bf16, kind="ExternalInput").ap()
    cosT = nc.dram_tensor("cosT", [DH, T], f32, kind="ExternalInput").ap()
    sinT = nc.dram_tensor("sinT", [DH, T], f32, kind="ExternalInput").ap()
    rot_t = nc.dram_tensor("rot_t", [DH, DH], bf16, kind="ExternalInput").ap()
    lnw = nc.dram_tensor("ln_w", [D], f32, kind="ExternalInput").ap()
    out_cT = nc.dram_tensor("out_colsT", [DL, T], f32,
                            kind="ExternalOutput").ap()

    Act = mybir.ActivationFunctionType
    Alu = mybir.AluOpType
    inv_sqrt_dh = 1.0 / math.sqrt(DH)
    MAGIC = 0x5F3759DF

    with tile.TileContext(nc) as tc, \
            tc.tile_pool(name="persist", bufs=1) as persist:
        # ---------------- long-lived tensors ----------------
        Q_sb = persist.tile([P, NH, T], bf16, tag="Q_sb")
        K_sb = persist.tile([P, NH, T], bf16, tag="K_sb")
        V_sb = persist.tile([P, NTS, DL], bf16, tag="V_sb")
        rcol_sb = persist.tile([P, NTS], f32, tag="rcol_sb")
        rrow_sb = persist.tile([1, T], f32, tag="rrow_sb")
        ones_bf = persist.tile([P, 1], bf16, tag="ones_bf")
        masks_sb = persist.tile([P, 4, 512], f32, tag="masks_sb")
        rot_sb = persist.tile([P, DH], bf16, tag="rot_sb")

        nc.gpsimd.dma_start(rot_sb[:], rot_t)
        nc.vector.memset(ones_bf[:], 1.0)
        warm_sb = persist.tile([P, 128], bf16, tag="warm_sb")
        nc.vector.memset(warm_sb[:], 0.0)
        nc.gpsimd.memset(masks_sb[:], 0.0)
        for r in range(4):
            # keep (0) where i - j >= 0 with i = 512*B + f, j = 128*J + p,
            # offset r = J - 4*B  ->  f - p - 128 r >= 0
            nc.gpsimd.affine_select(
                out=masks_sb[:, r, :], in_=masks_sb[:, r, :],
                pattern=[[1, 512]], channel_multiplier=-1, base=-128 * r,
                compare_op=Alu.is_ge, fill=NEG)

        with tc.tile_pool(name="dram", bufs=1, space="DRAM") as dram_pool:
            ag_shared = "Shared" if n_cores > 4 else "Local"
            ag_in = [[dram_pool.tile([DH, 512], bf16, tag=f"agi{h}_{b}",
                                     name=f"ag_in{h}_{b}")
                      for b in range(NIB)] for h in range(NH)]
            ag_out = [[dram_pool.tile([n_cores * DH, 512], bf16,
                                      addr_space=ag_shared, tag=f"ago{h}_{b}",
                                      name=f"ag_out{h}_{b}")
                       for b in range(NIB)] for h in range(NH)]

            # PE warmup: ~5us of back-to-back dummy matmuls so the HAM
            # clock gate opens before the real work arrives
            with tc.tile_pool(name="warm_ps", bufs=1, space="PSUM") as wmps:
                wps = wmps.tile([P, 128], f32, tag="wm")
                for _ in range(40):
                    nc.tensor.matmul(wps[:], warm_sb[:], warm_sb[:],
                                     start=True, stop=True)

            # ==== phases 1+2 interleaved per t-block: QKV projections,
            # attention, and the per-(head,block) all-gather — so the
            # collective stream starts early and hides under compute.
            with (
                tc.tile_pool(name="wqkv", bufs=1) as wpool,
                tc.tile_pool(name="cs_raw", bufs=1) as cspool,
                tc.tile_pool(name="xk", bufs=1) as xpool,
                tc.tile_pool(name="sq", bufs=4) as sqpool,
                tc.tile_pool(name="tmp1", bufs=4) as tmppool,
                tc.tile_pool(name="rbc1", bufs=2) as rbcpool,
                tc.tile_pool(name="pt", bufs=3) as ptpool,
                tc.tile_pool(name="fin", bufs=2) as finpool,
                tc.tile_pool(name="qk_ps", bufs=1, space="PSUM") as qkps,
                tc.tile_pool(name="v_ps", bufs=1, space="PSUM") as vps,
                tc.tile_pool(name="row_ps", bufs=2, space="PSUM") as rowps,
                tc.tile_pool(name="st_ps", bufs=1, space="PSUM") as stpool,
                tc.tile_pool(name="av_ps", bufs=1, space="PSUM") as avpool,
            ):
                lnw_sb = wpool.tile([P, KC], f32, tag="lnw")
                nc.sync.dma_start(lnw_sb[:], lnw.rearrange("(kc p) -> p kc", p=P))
                wq_sb = wpool.tile([P, KC, DL], bf16, tag="wq")
                wk_sb = wpool.tile([P, KC, DL], bf16, tag="wk")
                wv_sb = wpool.tile([P, KC, DL], bf16, tag="wv")
                # interleave weight / x^T loads so the first q/k matmul can
                # start as soon as wq + xk[0] have landed
                xk = [xpool.tile([P, T], bf16, tag=f"xk{kc}", name=f"xk{kc}")
                      for kc in range(KC)]
                nc.sync.dma_start(wq_sb[:], wq_t.rearrange("p (kc j) -> p kc j", j=DL))
                for kc in range(KC):
                    nc.sync.dma_start(xk[kc][:], xT[P * kc:P * (kc + 1), :])
                nc.sync.dma_start(wk_sb[:], wk_t.rearrange("p (kc j) -> p kc j", j=DL))
                nc.sync.dma_start(wv_sb[:], wv_t.rearrange("p (kc j) -> p kc j", j=DL))
                # fold ln_w into the projection weights (free-dim broadcast,
                # quarter granularity so the first matmuls unblock early)
                qn = max(1, KC // 4)
                for w in (wq_sb, wk_sb, wv_sb):
                    for q0 in range(0, KC, qn):
                        nc.vector.tensor_tensor(
                            w[:, q0:q0 + qn, :], w[:, q0:q0 + qn, :],
                            lnw_sb[:, q0:q0 + qn, None].to_broadcast(
                                (P, qn, DL)), Alu.mult)

                # cos/sin tables; r is folded in per block, in place
                cos_r = cspool.tile([P, T], f32, tag="cos")
                sin_r = cspool.tile([P, T], f32, tag="sin")
                nc.sync.dma_start(cos_r[:], cosT)
                nc.sync.dma_start(sin_r[:], sinT)

                for B in range(TB):
                    tb = slice(512 * B, 512 * B + 512)
                    # ---------- phase 1 for block B ----------
                    srow = rowps.tile([1, 512], f32, tag="row")
                    qps = qkps.tile([P, NH, 512], f32, tag="qk")
                    for kc in range(KC):
                        sq = sqpool.tile([P, 512], bf16, tag="sq")
                        nc.scalar.activation(sq[:], xk[kc][:, tb], Act.Square)
                        nc.tensor.matmul(srow[:], ones_bf[:], sq[:],
                                         start=(kc == 0), stop=(kc == KC - 1))
                        for h in range(NH):
                            hs = slice(DH * h, DH * (h + 1))
                            nc.tensor.matmul(qps[:, h, :], wq_sb[:, kc, hs],
                                             xk[kc][:, tb], start=(kc == 0),
                                             stop=(kc == KC - 1))
                    for h in range(NH):
                        nc.vector.tensor_copy(Q_sb[:, h, tb], qps[:, h, :])
                    # r = rsqrt(mean + eps): bit-trick seed + 2 Newton (DVE)
                    rr = rrow_sb[0:1, tb]
                    mrow = tmppool.tile([1, 512], f32, tag="mrow")
                    nc.vector.tensor_scalar(mrow[:], srow[:], 1.0 / D, EPS,
                                            Alu.mult, Alu.add)
                    ri = tmppool.tile([1, 512], i32, tag="ri")
                    nc.vector.tensor_scalar(ri[:], mrow[:].bitcast(i32), 1, None,
                                            Alu.arith_shift_right)
                    nc.vector.tensor_scalar(ri[:], ri[:], -1, MAGIC,
                                            Alu.mult, Alu.add)
                    rrv = ri[:].bitcast(f32)
                    tn = tmppool.tile([1, 512], f32, tag="tn")
                    nc.vector.tensor_tensor(tn[:], rrv, rrv, Alu.mult)
                    nc.vector.tensor_tensor(tn[:], tn[:], mrow[:], Alu.mult)
                    nc.vector.tensor_scalar(tn[:], tn[:], -0.5, 1.5,
                                            Alu.mult, Alu.add)
                    nc.vector.tensor_tensor(rrv, rrv, tn[:], Alu.mult)
                    nc.vector.tensor_tensor(tn[:], rrv, rrv, Alu.mult)
                    nc.vector.tensor_tensor(tn[:], tn[:], mrow[:], Alu.mult)
                    nc.vector.tensor_scalar(tn[:], tn[:], -0.5, 1.5,
                                            Alu.mult, Alu.add)
                    nc.vector.tensor_tensor(rr, rrv, tn[:], Alu.mult)
                    rbc = rbcpool.tile([P, 512], f32, tag="rbc")
                    nc.gpsimd.partition_broadcast(rbc[:], rr)
                    for s in range(4):
                        i = 4 * B + s
                        nc.gpsimd.dma_start(
                            out=rcol_sb[:, i:i + 1],
                            in_=rrow_sb[0:1, 512 * B + 128 * s:
                                        512 * B + 128 * (s + 1)])
                    nc.vector.tensor_tensor(cos_r[:, tb], cos_r[:, tb], rbc[:], Alu.mult)
                    nc.vector.tensor_tensor(sin_r[:, tb], sin_r[:, tb], rbc[:], Alu.mult)
                    # K pass (reuses the same psum slot after the Q drain)
                    kps = qkps.tile([P, NH, 512], f32, tag="qk")
                    for kc in range(KC):
                        for h in range(NH):
                            hs = slice(DH * h, DH * (h + 1))
                            nc.tensor.matmul(kps[:, h, :], wk_sb[:, kc, hs],
                                             xk[kc][:, tb], start=(kc == 0),
                                             stop=(kc == KC - 1))
                    for h in range(NH):
                        nc.vector.tensor_copy(K_sb[:, h, tb], kps[:, h, :])
                    # V pass, one 512-row tile (1 psum bank) at a time
                    for ts in range(4):
                        i = 4 * B + ts
                        vp = vps.tile([P, 512], f32, tag="v")
                        for kc in range(KC):
                            nc.tensor.matmul(vp[:, :DL],
                                             xk[kc][:, 512 * B + P * ts:
                                                    512 * B + P * (ts + 1)],
                                             wv_sb[:, kc, :], start=(kc == 0),
                                             stop=(kc == KC - 1))
                        nc.vector.tensor_copy(V_sb[:, i, :], vp[:, :DL])
                        nc.vector.tensor_scalar_mul(V_sb[:, i, :], V_sb[:, i, :],
                                                    rcol_sb[:, i:i + 1])
                    # RoPE in place on SBUF (r enters via the scaled tables)
                    for buf in (Q_sb, K_sb):
                        for h in range(NH):
                            qs = tmppool.tile([P, 512], bf16, tag="qs")
                            nc.vector.tensor_tensor(qs[:], buf[:, h, tb],
                                                    sin_r[:, tb], Alu.mult)
                            rps = vps.tile([P, 512], f32, tag="v")
                            nc.tensor.matmul(rps[:], rot_sb[:], qs[:],
                                             start=True, stop=True)
                            nc.vector.tensor_tensor(buf[:, h, tb], buf[:, h, tb],
                                                    cos_r[:, tb], Alu.mult)
                            nc.vector.tensor_tensor(buf[:, h, tb], buf[:, h, tb],
                                                    rps[:], Alu.add)
                    # ---------- phase 2 for block B (both heads) ----------
                    ib = tb
                    for h in range(NH):
                        hs = slice(DH * h, DH * (h + 1))
                        av = avpool.tile([P, 512], f32, tag="av")
                        ssum = rowps.tile([1, 512], f32, tag="row")
                        Jmax = 4 * B + 3
                        for Jp in range(0, Jmax + 1, 2):
                            st = stpool.tile([P, 2, 512], f32, tag="st")
                            pt = ptpool.tile([P, 2, 512], bf16, tag="pt")
                            for gi in range(2):
                                J = Jp + gi
                                nc.tensor.matmul(st[:, gi, :],
                                                 K_sb[:, h, P * J:P * (J + 1)],
                                                 Q_sb[:, h, ib],
                                                 start=True, stop=True)
                                if J // 4 == B:
                                    nc.vector.tensor_tensor(
                                        st[:, gi, :], st[:, gi, :],
                                        masks_sb[:, J % 4, :], Alu.add)
                            nc.scalar.activation(pt[:], st[:], Act.Exp,
                                                 scale=inv_sqrt_dh)
                            for gi in range(2):
                                J = Jp + gi
                                nc.tensor.matmul(av[:], V_sb[:, J, hs],
                                                 pt[:, gi, :], start=(J == 0),
                                                 stop=(J == Jmax))
                                nc.tensor.matmul(ssum[:], ones_bf[:],
                                                 pt[:, gi, :], start=(J == 0),
                                                 stop=(J == Jmax))
                        rinv = finpool.tile([1, 512], f32, tag="rinv")
                        nc.vector.reciprocal_approx_fast(rinv[:], ssum[:])
                        rb = finpool.tile([P, 512], f32, tag="rb")
                        nc.gpsimd.partition_broadcast(rb[:], rinv[:])
                        att = finpool.tile([P, 512], bf16, tag="att")
                        nc.vector.tensor_tensor(att[:], av[:], rb[:], Alu.mult)
                        nc.sync.dma_start(ag_in[h][B][:], att[:])
                        nc.gpsimd.collective_compute(
                            "AllGather", Alu.bypass,
                            replica_groups=[list(range(n_cores))],
                            ins=[ag_in[h][B][:].opt()],
                            outs=[ag_out[h][B][:].opt()])

            # ================= phase 4: output projection =================
            with (
                tc.tile_pool(name="wo", bufs=1) as wopool,
                tc.tile_pool(name="ag_sb", bufs=20) as agpool,
                tc.tile_pool(name="xc", bufs=4) as xcpool,
                tc.tile_pool(name="osb", bufs=4) as opool,
                tc.tile_pool(name="o_ps", bufs=2, space="PSUM") as ops,
            ):
                wo_sb = wopool.tile([P, KC, DL], bf16, tag="wo")
                nc.sync.dma_start(wo_sb[:], wo_t.rearrange("p (kc j) -> p kc j", j=DL))
                for B in range(NIB):
                    sl = slice(512 * B, 512 * B + 512)
                    ags = []
                    for kc in range(KC):
                        h_idx, c_idx = divmod(kc, KC // NH)
                        agt = agpool.tile([P, 512], bf16, tag="ag",
                                          name=f"ag{kc}_{B}")
                        eng = nc.sync if kc % 2 == 0 else nc.gpsimd
                        eng.dma_start(
                            agt[:],
                            ag_out[h_idx][B][P * c_idx:P * (c_idx + 1), :])
                        ags.append(agt)
                    for js in range(DL // P):
                        om = ops.tile([P, 512], f32, tag="om",
                                      name=f"om{js}_{B}")
                        for kc in range(KC):
                            nc.tensor.matmul(
                                om[:], wo_sb[:, kc, P * js:P * (js + 1)],
                                ags[kc][:], start=(kc == 0),
                                stop=(kc == KC - 1))
                        xct = xcpool.tile([P, 512], f32, tag="xct")
                        nc.sync.dma_start(xct[:], xct_in[P * js:P * (js + 1), sl])
                        osb = opool.tile([P, 512], f32, tag="osb")
                        nc.vector.tensor_tensor(osb[:], om[:], xct[:], Alu.add)
                        nc.sync.dma_start(out_cT[P * js:P * (js + 1), sl],
                                          osb[:])

    nc.compile()
    return nc


# --------------------------------------------------------------------------
# host-side prep / entry point
# --------------------------------------------------------------------------
def prepare_inputs(x, cos, sin, ln_w, Wq, Wk, Wv, Wo, n_cores, heads_per_core):
    import ml_dtypes
    bf16 = ml_dtypes.bfloat16
    DH = 128
    DL = heads_per_core * DH
    x = np.ascontiguousarray(np.asarray(x, dtype=np.float32))
    cos = np.asarray(cos, dtype=np.float32)
    sin = np.asarray(sin, dtype=np.float32)
    ln_w = np.ascontiguousarray(np.asarray(ln_w, dtype=np.float32))
    xT = np.ascontiguousarray(x.T.astype(bf16))
    cosT = np.ascontiguousarray(cos.T)
    sinT = np.ascontiguousarray(sin.T)
    R = np.zeros((DH, DH), dtype=np.float32)
    R[np.arange(64), np.arange(64) + 64] = -1.0
    R[np.arange(64) + 64, np.arange(64)] = 1.0
    rot_t = np.ascontiguousarray(R.T.astype(bf16))
    # AllGather chunk order: head-major, then source core; each chunk is the
    # 128 att columns (global j = DL*c' + DH*h + d) that core c' / head h sent.
    perm = np.concatenate([
        DL * cp + DH * h + np.arange(DH)
        for h in range(heads_per_core) for cp in range(n_cores)
    ])
    D = x.shape[1]
    KC = D // DH

    def pretile(wT):
        # (D, DL) -> SBUF layout [P, KC*DL]: element (p, kc, j) = wT[128 kc + p, j]
        return np.ascontiguousarray(
            wT.reshape(KC, DH, DL).transpose(1, 0, 2).reshape(DH, KC * DL)
            .astype(bf16))

    in_maps = []
    for c in range(n_cores):
        cols = slice(c * DL, (c + 1) * DL)
        woT = np.asarray(Wo, np.float32)[cols, :].T  # (D, DL)
        in_maps.append({
            "xT": xT,
            "x_colsT": np.ascontiguousarray(x[:, cols].T),
            "wq_t": pretile(np.asarray(Wq, np.float32)[cols, :].T),
            "wk_t": pretile(np.asarray(Wk, np.float32)[cols, :].T),
            "wv_t": pretile(np.asarray(Wv, np.float32)[cols, :].T),
            "wo_t": pretile(woT[perm, :]),
            "cosT": cosT,
            "sinT": sinT,
            "rot_t": rot_t,
            "ln_w": ln_w,
        })
    return in_maps


_NC_CACHE = {}


def kernel(x, cos, sin, attention_mask, ln_w, Wq, Wk, Wv, Wo,
           _trace=False, _trace_cores=None):
    from concourse.bass_utils import run_bass_kernel_spmd

    cfg = CFG_FULL
    key = tuple(sorted(cfg.items()))
    if key not in _NC_CACHE:
        _NC_CACHE[key] = build_nc(**cfg)
    nc = _NC_CACHE[key]
    n_cores = cfg["n_cores"]
    in_maps = prepare_inputs(x, cos, sin, ln_w, Wq, Wk, Wv, Wo,
                             n_cores, cfg["heads_per_core"])
    res = run_bass_kernel_spmd(nc, in_maps, core_ids=list(range(n_cores)),
                               trace=_trace, trace_cores=_trace_cores)
    out = np.concatenate(
        [res.results[c]["out_colsT"].T for c in range(n_cores)], axis=1)
    kernel.last_result = res
    return out



# revision 3
# speedup vs baseline: 1.0391x; 1.0391x over previous
"""Trainium2 Bass kernel for a dense-transformer attention block.

Reference semantics (T=2048, D=2048, 16 heads, d_h=128):
    h = RMSNorm(x) * ln_w
    q,k,v = h @ W{q,k,v}.T  -> (n_h, T, d_h);  RoPE(q, k)
    att = softmax(causal(q k^T / sqrt(d_h))) @ v
    out = x + att @ Wo.T          (attention_mask is all-ones per spec)

Distribution: head-parallel over 8 cores (2 heads/core) for QKV+attention;
the output is SEQUENCE-sharded: one AllToAll redistributes att^T so each
core holds all 2048 att rows for its own 256 output timesteps, then each
core runs the FULL Wo against its timestep slice.  This moves ~8x fewer
collective bytes than all-gathering att.

Per-core phases:
  prelude  RMSNorm row scales r[t]: Act-engine Square with accum_out over a
           row-major copy of x (no PE work), bit-trick rsqrt Newton on DVE
           in [128,16] column layout; r folded into the RoPE tables.
  per 512-row block B:
    QKV    projections for its 2 heads (bf16 matmuls, ln_w folded into
           weights on host).  RoPE runs on DVE with a sign-folded sin table
           and a partition-half-swap SBUF DMA (no PE rotate matmul).
    att    causal attention, scores transposed (S^T[j,i]); i-range sliced
           to skip fully-masked diagonal work; softmax row-sums accumulate
           on the PE via a ones-vector matmul; exp on ScalarE.
    a2a-in att^T chunks stored to the AllToAll source buffer.
  A2A      one AllToAll (att^T, bf16) + a tiny warmup AllToAll at t=0.
  Wo       out[i,:] = att_all^T @ Wo streamed kc-chunk-wise (weights DMA'd
           just-in-time), + residual from f32 x rows; out is [256,2048] f32.
Host assembles out = concat(out_rows, axis=0).
"""

import math

import numpy as np

EPS = 1e-5
NEG = -1.0e30

CFG_FULL = dict(T=2048, D=2048, n_cores=8, heads_per_core=2)


# --------------------------------------------------------------------------
# device program
# --------------------------------------------------------------------------
def build_nc(T, D, n_cores, heads_per_core):
    import concourse.mybir as mybir
    import concourse.tile as tile
    from concourse import bacc

    DH = 128                      # head dim (hard-wired into layout)
    P = 128                       # partitions
    NH = heads_per_core
    DL = NH * DH                  # local width (q/k/v columns per core)
    KC = D // P                   # 128-contraction chunks over d_model
    TB = T // 512                 # 512-wide t blocks
    NTS = T // P                  # 128-wide t subtiles
    TS_C = T // n_cores           # output timesteps per core (256)
    RG = 4                        # Wo column groups of 512
    f32 = mybir.dt.float32
    bf16 = mybir.dt.bfloat16
    i32 = mybir.dt.int32

    nc = bacc.Bacc("TRN2", target_bir_lowering=False, debug=False,
                   num_devices=n_cores)

    # ---- I/O ----
    xT = nc.dram_tensor("xT", [D, T], bf16, kind="ExternalInput").ap()
    x_rows = nc.dram_tensor("x_rows", [T, D], bf16, kind="ExternalInput").ap()
    x_seq = nc.dram_tensor("x_seq", [TS_C, D], f32, kind="ExternalInput").ap()
    # weight tensors arrive host-pretiled in SBUF layout
    wq_t = nc.dram_tensor("wq_t", [P, KC * DL], bf16, kind="ExternalInput").ap()
    wk_t = nc.dram_tensor("wk_t", [P, KC * DL], bf16, kind="ExternalInput").ap()
    wv_t = nc.dram_tensor("wv_t", [P, KC * DL], bf16, kind="ExternalInput").ap()
    wo_t = nc.dram_tensor("wo_t", [P, KC * RG * 512], bf16,
                          kind="ExternalInput").ap()
    cosT = nc.dram_tensor("cosT", [DH, T], f32, kind="ExternalInput").ap()
    sinmT = nc.dram_tensor("sinmT", [DH, T], f32, kind="ExternalInput").ap()
    out_rows = nc.dram_tensor("out_rows", [TS_C, D], f32,
                              kind="ExternalOutput").ap()

    Act = mybir.ActivationFunctionType
    Alu = mybir.AluOpType
    inv_sqrt_dh = 1.0 / math.sqrt(DH)
    MAGIC = 0x5F3759DF

    with tile.TileContext(nc) as tc, \
            tc.tile_pool(name="persist", bufs=1) as persist:
        # ---------------- long-lived tensors ----------------
        Q_sb = persist.tile([P, NH, T], bf16, tag="Q_sb")
        K_sb = persist.tile([P, NH, T], bf16, tag="K_sb")
        V_sb = persist.tile([P, NTS, DL], bf16, tag="V_sb")
        rcol_sb = persist.tile([P, NTS], f32, tag="rcol_sb")
        rrow_sb = persist.tile([1, T], f32, tag="rrow_sb")
        ones_bf = persist.tile([P, 1], bf16, tag="ones_bf")
        mask_sb = persist.tile([P, P], f32, tag="mask_sb")
        warm_sb = persist.tile([P, 128], bf16, tag="warm_sb")

        nc.vector.memset(ones_bf[:], 1.0)
        nc.vector.memset(warm_sb[:], 0.0)
        nc.gpsimd.memset(mask_sb[:], 0.0)
        # keep (0) where j <= i within the diagonal 128x128 block:
        # j = base + p, i = base + f  ->  keep f - p >= 0
        nc.gpsimd.affine_select(
            out=mask_sb[:], in_=mask_sb[:],
            pattern=[[1, P]], channel_multiplier=-1, base=0,
            compare_op=Alu.is_ge, fill=NEG)

        with tc.tile_pool(name="dram", bufs=1, space="DRAM") as dram_pool:
            a2a_in = dram_pool.tile([n_cores, DL, TS_C], bf16, tag="a2ai",
                                    name="a2a_in")
            a2a_out = dram_pool.tile([n_cores, DL, TS_C], bf16, tag="a2ao",
                                     name="a2a_out")
            dummy_i = dram_pool.tile([n_cores, 128], bf16, tag="dmi",
                                     name="dummy_i")
            dummy_o = dram_pool.tile([n_cores, 128], bf16, tag="dmo",
                                     name="dummy_o")

            # PE warmup: ~7us of back-to-back dummy matmuls so the HAM
            # clock gate opens before the real work arrives
            with tc.tile_pool(name="warm_ps", bufs=1, space="PSUM") as wmps:
                wps = wmps.tile([P, 128], f32, tag="wm")
                for _ in range(32):
                    nc.tensor.matmul(wps[:], warm_sb[:], warm_sb[:],
                                     start=True, stop=True)

            with (
                tc.tile_pool(name="wqkv", bufs=1) as wpool,
                tc.tile_pool(name="cs_raw", bufs=1) as cspool,
                tc.tile_pool(name="xk", bufs=1) as xpool,
                tc.tile_pool(name="xrow", bufs=3) as xrpool,
                tc.tile_pool(name="sqs", bufs=2) as sqpool,
                tc.tile_pool(name="rtmp", bufs=1) as rpool,
                tc.tile_pool(name="rope", bufs=4) as ropool,
                tc.tile_pool(name="ptp", bufs=3) as ptpool,
                tc.tile_pool(name="fin", bufs=2) as finpool,
                tc.tile_pool(name="proj_ps", bufs=3, space="PSUM") as pps,
                tc.tile_pool(name="st_ps", bufs=2, space="PSUM") as stps,
                tc.tile_pool(name="av_ps", bufs=1, space="PSUM") as avps,
                tc.tile_pool(name="row_ps", bufs=1, space="PSUM") as rowps,
            ):
                # ---- initial loads ----
                wq_sb = wpool.tile([P, KC, DL], bf16, tag="wq")
                wk_sb = wpool.tile([P, KC, DL], bf16, tag="wk")
                wv_sb = wpool.tile([P, KC, DL], bf16, tag="wv")
                xk = [xpool.tile([P, T], bf16, tag=f"xk{kc}", name=f"xk{kc}")
                      for kc in range(KC)]
                nc.sync.dma_start(wq_sb[:], wq_t.rearrange("p (kc j) -> p kc j", j=DL))
                for kc in range(KC):
                    nc.sync.dma_start(xk[kc][:], xT[P * kc:P * (kc + 1), :])
                nc.sync.dma_start(wk_sb[:], wk_t.rearrange("p (kc j) -> p kc j", j=DL))
                nc.sync.dma_start(wv_sb[:], wv_t.rearrange("p (kc j) -> p kc j", j=DL))
                cos_r = cspool.tile([P, T], f32, tag="cos")
                sinm_r = cspool.tile([P, T], f32, tag="sin")
                nc.scalar.dma_start(cos_r[:], cosT)
                nc.scalar.dma_start(sinm_r[:], sinmT)

                # warm up the collective path while compute is starting
                nc.sync.dma_start(dummy_i[:], warm_sb[:n_cores, :])
                nc.gpsimd.collective_compute(
                    "AllToAll", Alu.bypass,
                    replica_groups=[list(range(n_cores))],
                    ins=[dummy_i[:].opt()], outs=[dummy_o[:].opt()])

                # ---- RMSNorm row scales: r = rsqrt(mean(x^2) + eps) ----
                # Square+row-sum on the Act engine over row-major x tiles.
                rs_raw = rpool.tile([P, NTS], f32, tag="rs_raw")
                for ch in range(NTS):
                    xr = xrpool.tile([P, D], bf16, tag="xr", name=f"xr{ch}")
                    nc.scalar.dma_start(xr[:], x_rows[P * ch:P * (ch + 1), :])
                    scr = sqpool.tile([P, D], bf16, tag="scr")
                    nc.scalar.activation(scr[:], xr[:], Act.Square,
                                         accum_out=rs_raw[:, ch:ch + 1])
                # r = rsqrt(sum/D + eps): bit-trick seed + 2 Newton (DVE)
                mcol = rpool.tile([P, NTS], f32, tag="mcol")
                nc.vector.tensor_scalar(mcol[:], rs_raw[:], 1.0 / D, EPS,
                                        Alu.mult, Alu.add)
                ri = rpool.tile([P, NTS], i32, tag="ri")
                nc.vector.tensor_scalar(ri[:], mcol[:].bitcast(i32), 1, None,
                                        Alu.arith_shift_right)
                nc.vector.tensor_scalar(ri[:], ri[:], -1, MAGIC,
                                        Alu.mult, Alu.add)
                rrv = ri[:].bitcast(f32)
                tn = rpool.tile([P, NTS], f32, tag="tn")
                for _ in range(2):
                    nc.vector.tensor_tensor(tn[:], rrv, rrv, Alu.mult)
                    nc.vector.tensor_tensor(tn[:], tn[:], mcol[:], Alu.mult)
                    nc.vector.tensor_scalar(tn[:], tn[:], -0.5, 1.5,
                                            Alu.mult, Alu.add)
                    nc.vector.tensor_tensor(rrv, rrv, tn[:], Alu.mult)
                nc.vector.tensor_copy(rcol_sb[:], rrv)
                # spread r back to a row vector and fold into the tables
                for ch in range(NTS):
                    nc.gpsimd.dma_start(rrow_sb[0:1, P * ch:P * (ch + 1)],
                                        rcol_sb[:, ch:ch + 1])
                rbc = rpool.tile([P, T], f32, tag="rbc")
                nc.gpsimd.partition_broadcast(rbc[:], rrow_sb[0:1, :])
                nc.vector.tensor_tensor(cos_r[:], cos_r[:], rbc[:], Alu.mult)
                nc.vector.tensor_tensor(sinm_r[:], sinm_r[:], rbc[:], Alu.mult)

                # ---- per-block QKV + attention, software-pipelined ----
                def qkv_block(B):
                    tb = slice(512 * B, 512 * B + 512)
                    for qk, (w_sb, dst) in enumerate(((wq_sb, Q_sb),
                                                      (wk_sb, K_sb))):
                        for h in range(NH):
                            hs = slice(DH * h, DH * (h + 1))
                            qp = pps.tile([P, 512], f32, tag="proj",
                                          name=f"p{B}_{qk}_{h}")
                            for kc in range(KC):
                                nc.tensor.matmul(qp[:], w_sb[:, kc, hs],
                                                 xk[kc][:, tb],
                                                 start=(kc == 0),
                                                 stop=(kc == KC - 1))
                            # RoPE on DVE: dst = qp*cos_r + swap64(qp*sinm_r)
                            tmp = ropool.tile([P, 512], bf16, tag="tmp")
                            tmp2 = ropool.tile([P, 512], bf16, tag="tmp2")
                            nc.vector.tensor_tensor(tmp[:], qp[:],
                                                    sinm_r[:, tb], Alu.mult)
                            nc.sync.dma_start(tmp2[0:64, :], tmp[64:128, :])
                            nc.sync.dma_start(tmp2[64:128, :], tmp[0:64, :])
                            nc.vector.tensor_tensor(dst[:, h, tb], qp[:],
                                                    cos_r[:, tb], Alu.mult)
                            nc.vector.tensor_tensor(dst[:, h, tb],
                                                    dst[:, h, tb], tmp2[:],
                                                    Alu.add)
                    for ts in range(4):
                        i = 4 * B + ts
                        tsl = slice(512 * B + P * ts, 512 * B + P * (ts + 1))
                        vp = pps.tile([P, 512], f32, tag="proj",
                                      name=f"pv{B}_{ts}")
                        for kc in range(KC):
                            nc.tensor.matmul(vp[:, :DL], xk[kc][:, tsl],
                                             wv_sb[:, kc, :], start=(kc == 0),
                                             stop=(kc == KC - 1))
                        nc.vector.tensor_scalar_mul(V_sb[:, i, :], vp[:, :DL],
                                                    rcol_sb[:, i:i + 1])

                def att_block(B):
                    ib = slice(512 * B, 512 * B + 512)
                    for h in range(NH):
                        hs = slice(DH * h, DH * (h + 1))
                        av = avps.tile([P, 512], f32, tag="av",
                                       name=f"av{B}_{h}")
                        ssum = rowps.tile([1, 512], f32, tag="row",
                                          name=f"ss{B}_{h}")
                        Jmax = 4 * B + 3
                        for J in range(Jmax + 1):
                            r = J - 4 * B  # >=0 on the diagonal 512-block
                            lo = max(0, 128 * r)  # live i-range start
                            isl = slice(512 * B + lo, 512 * B + 512)
                            st = stps.tile([P, 512], f32, tag="st",
                                           name=f"st{B}_{h}_{J}")
                            nc.tensor.matmul(st[:, lo:],
                                             K_sb[:, h, P * J:P * (J + 1)],
                                             Q_sb[:, h, isl],
                                             start=True, stop=True)
                            if r >= 0:
                                nc.vector.tensor_tensor(
                                    st[:, lo:lo + P], st[:, lo:lo + P],
                                    mask_sb[:], Alu.add)
                            pt = ptpool.tile([P, 512], bf16, tag="pt")
                            nc.scalar.activation(pt[:, lo:], st[:, lo:],
                                                 Act.Exp, scale=inv_sqrt_dh)
                            nc.tensor.matmul(av[:, lo:], V_sb[:, J, hs],
                                             pt[:, lo:], start=(J == 0),
                                             stop=(J == Jmax),
                                             skip_group_check=True)
                            nc.tensor.matmul(ssum[:, lo:], ones_bf[:],
                                             pt[:, lo:], start=(J == 0),
                                             stop=(J == Jmax),
                                             skip_group_check=True)
                        rinv = finpool.tile([1, 512], f32, tag="rinv")
                        nc.vector.reciprocal_approx_fast(rinv[:], ssum[:])
                        rb = finpool.tile([P, 512], f32, tag="rb")
                        nc.gpsimd.partition_broadcast(rb[:], rinv[:])
                        att = finpool.tile([P, 512], bf16, tag="att")
                        nc.vector.tensor_tensor(att[:], av[:], rb[:], Alu.mult)
                        # store to the AllToAll source: dest core 2B+c' gets
                        # i-window att[:, 256c' : 256c'+256]
                        nc.sync.dma_start(
                            a2a_in[2 * B:2 * B + 2, DH * h:DH * (h + 1), :]
                            .rearrange("c p i -> p c i"),
                            att[:])

                qkv_block(0)
                qkv_block(1)
                att_block(0)
                qkv_block(2)
                att_block(1)
                qkv_block(3)
                att_block(2)
                att_block(3)

                nc.gpsimd.collective_compute(
                    "AllToAll", Alu.bypass,
                    replica_groups=[list(range(n_cores))],
                    ins=[a2a_in[:].opt()], outs=[a2a_out[:].opt()])

            # ================= output projection =================
            av_flat = a2a_out[:].rearrange("c d i -> (c d) i")
            with (
                tc.tile_pool(name="attall", bufs=1) as apool,
                tc.tile_pool(name="wo", bufs=4) as wopool,
                tc.tile_pool(name="xc", bufs=2) as xcpool,
                tc.tile_pool(name="osb", bufs=2) as opool,
                tc.tile_pool(name="o_ps", bufs=8, space="PSUM") as ops,
            ):
                att_all = apool.tile([P, KC, TS_C], bf16, tag="att_all")
                wo_view = wo_t.rearrange("p (kc rg j) -> p kc rg j",
                                         kc=KC, rg=RG)
                wo_chunks = []
                for kc in range(KC):
                    nc.sync.dma_start(att_all[:, kc, :],
                                      av_flat[P * kc:P * (kc + 1), :])
                    woc = wopool.tile([P, RG, 512], bf16, tag="woc",
                                      name=f"woc{kc}")
                    nc.scalar.dma_start(woc[:], wo_view[:, kc, :, :])
                    wo_chunks.append(woc)
                outp = [ops.tile([P, 512], f32, tag="om", name=f"om{t}")
                        for t in range(8)]
                for kc in range(KC):
                    for isub in range(2):
                        for rg in range(RG):
                            nc.tensor.matmul(
                                outp[4 * isub + rg][:],
                                att_all[:, kc, P * isub:P * (isub + 1)],
                                wo_chunks[kc][:, rg, :],
                                start=(kc == 0), stop=(kc == KC - 1))
                for isub in range(2):
                    for rg in range(RG):
                        rsl = slice(512 * rg, 512 * (rg + 1))
                        xct = xcpool.tile([P, 512], f32, tag="xct")
                        nc.sync.dma_start(
                            xct[:], x_seq[P * isub:P * (isub + 1), rsl])
                        osb = opool.tile([P, 512], f32, tag="osb")
                        nc.vector.tensor_tensor(osb[:], outp[4 * isub + rg][:],
                                                xct[:], Alu.add)
                        nc.sync.dma_start(
                            out_rows[P * isub:P * (isub + 1), rsl], osb[:])

    nc.compile()
    return nc


# --------------------------------------------------------------------------
# host-side prep / entry point
# --------------------------------------------------------------------------
def prepare_inputs(x, cos, sin, ln_w, Wq, Wk, Wv, Wo, n_cores, heads_per_core):
    import ml_dtypes
    bf16 = ml_dtypes.bfloat16
    DH = 128
    DL = heads_per_core * DH
    x = np.ascontiguousarray(np.asarray(x, dtype=np.float32))
    T, D = x.shape
    KC = D // DH
    TS_C = T // n_cores
    cosT = np.ascontiguousarray(np.asarray(cos, np.float32).T)
    sinmT = np.asarray(sin, np.float32).T.copy()
    sinmT[64:, :] *= -1.0  # sign fold for the rotate-half swap trick
    sinmT = np.ascontiguousarray(sinmT)
    lnw = np.asarray(ln_w, np.float32)
    xT = np.ascontiguousarray(x.T.astype(bf16))
    x_rows = np.ascontiguousarray(x.astype(bf16))

    def pretile_qkv(W, cols):
        # rows j of W (out dims), ln_w folded; SBUF layout [P, KC*DL]
        arr = (np.asarray(W, np.float32)[cols, :] * lnw[None, :]).T  # (D, DL)
        return np.ascontiguousarray(
            arr.reshape(KC, DH, DL).transpose(1, 0, 2).reshape(DH, KC * DL)
            .astype(bf16))

    # Wo full, pretiled [P, KC*RG*512]: element (p, kc, rg, j) = Wo.T[128kc+p, 512rg+j]
    woT = np.asarray(Wo, np.float32).T  # (D, D) = (d_in, d_out)
    wo_t = np.ascontiguousarray(
        woT.reshape(KC, DH, 4, 512).transpose(1, 0, 2, 3)
        .reshape(DH, KC * 4 * 512).astype(bf16))

    in_maps = []
    for c in range(n_cores):
        cols = slice(c * DL, (c + 1) * DL)
        rows = slice(c * TS_C, (c + 1) * TS_C)
        in_maps.append({
            "xT": xT,
            "x_rows": x_rows,
            "x_seq": np.ascontiguousarray(x[rows, :]),
            "wq_t": pretile_qkv(Wq, cols),
            "wk_t": pretile_qkv(Wk, cols),
            "wv_t": pretile_qkv(Wv, cols),
            "wo_t": wo_t,
            "cosT": cosT,
            "sinmT": sinmT,
        })
    return in_maps


_NC_CACHE = {}


def kernel(x, cos, sin, attention_mask, ln_w, Wq, Wk, Wv, Wo,
           _trace=False, _trace_cores=None):
    from concourse.bass_utils import run_bass_kernel_spmd

    cfg = CFG_FULL
    key = tuple(sorted(cfg.items()))
    if key not in _NC_CACHE:
        _NC_CACHE[key] = build_nc(**cfg)
    nc = _NC_CACHE[key]
    n_cores = cfg["n_cores"]
    in_maps = prepare_inputs(x, cos, sin, ln_w, Wq, Wk, Wv, Wo,
                             n_cores, cfg["heads_per_core"])
    res = run_bass_kernel_spmd(nc, in_maps, core_ids=list(range(n_cores)),
                               trace=_trace, trace_cores=_trace_cores)
    out = np.concatenate(
        [res.results[c]["out_rows"] for c in range(n_cores)], axis=0)
    kernel.last_result = res
    return out


# revision 24
# speedup vs baseline: 1.1053x; 1.0637x over previous
"""Trainium2 Bass kernel for a dense-transformer attention block.

Reference semantics (T=2048, D=2048, 16 heads, d_h=128):
    h = RMSNorm(x) * ln_w
    q,k,v = h @ W{q,k,v}.T  -> (n_h, T, d_h);  RoPE(q, k)
    att = softmax(causal(q k^T / sqrt(d_h))) @ v
    out = x + att @ Wo.T          (attention_mask is all-ones per spec)

Distribution: head-parallel over 8 cores (2 heads/core) for QKV+attention;
the output is SEQUENCE-sharded: AllToAlls redistribute att^T so each core
holds all 2048 att rows for its own 256 output timesteps, then each core
runs the FULL Wo against its timestep slice.  This moves ~8x fewer
collective bytes than all-gathering att.  The AllToAll is split by head so
the first one overlaps the last attention block, and the Wo contraction is
ordered even-kc (head-0 rows) first so it can start before the second
AllToAll lands.

Per-core phases:
  prelude  RMSNorm row scales r[t]: Act-engine Square with accum_out over a
           row-major copy of x (no PE work), bit-trick rsqrt Newton on DVE
           in [128,8] column layout, two halves so early blocks unblock
           early; r folded into the RoPE tables (loaded bf16, scaled f32).
  per 512-row block B:
    QKV    projections for its 2 heads (bf16 matmuls, ln_w folded into
           weights on host).  RoPE runs on DVE with a sign-folded sin table
           and a partition-half-swap SBUF DMA (no PE rotate matmul).
    att    causal attention, scores transposed (S^T[j,i]); i-range sliced
           to skip fully-masked diagonal work; softmax row-sums accumulate
           on the PE via a ones-vector matmul; exp on ScalarE.
  Wo       out[i,:] = att_all^T @ Wo streamed kc-chunk-wise (weights DMA'd
           during attention on spread queues), + residual from f32 x rows.
Host assembles out = concat(out_rows, axis=0).
"""

import math

import numpy as np

EPS = 1e-5
NEG = -1.0e30

CFG_FULL = dict(T=2048, D=2048, n_cores=8, heads_per_core=2)


# --------------------------------------------------------------------------
# device program
# --------------------------------------------------------------------------
def build_nc(T, D, n_cores, heads_per_core):
    import concourse.mybir as mybir
    import concourse.tile as tile
    from concourse import bacc

    DH = 128                      # head dim (hard-wired into layout)
    P = 128                       # partitions
    NH = heads_per_core
    DL = NH * DH                  # local width (q/k/v columns per core)
    KC = D // P                   # 128-contraction chunks over d_model
    TB = T // 512                 # 512-wide t blocks
    NTS = T // P                  # 128-wide t subtiles
    TS_C = T // n_cores           # output timesteps per core (256)
    RG = 4                        # Wo column groups of 512
    f32 = mybir.dt.float32
    bf16 = mybir.dt.bfloat16
    i32 = mybir.dt.int32

    nc = bacc.Bacc("TRN2", target_bir_lowering=False, debug=False,
                   num_devices=n_cores)

    # ---- I/O ----
    xT = nc.dram_tensor("xT", [D, T], bf16, kind="ExternalInput").ap()
    x_rows = nc.dram_tensor("x_rows", [T, D], bf16, kind="ExternalInput").ap()
    x_seq = nc.dram_tensor("x_seq", [TS_C, D], f32, kind="ExternalInput").ap()
    # weight tensors arrive host-pretiled in SBUF layout
    wq_t = nc.dram_tensor("wq_t", [P, KC * DL], bf16, kind="ExternalInput").ap()
    wk_t = nc.dram_tensor("wk_t", [P, KC * DL], bf16, kind="ExternalInput").ap()
    wv_t = nc.dram_tensor("wv_t", [P, KC * DL], bf16, kind="ExternalInput").ap()
    wo_t = nc.dram_tensor("wo_t", [P, KC * RG * 512], bf16,
                          kind="ExternalInput").ap()
    cosT = nc.dram_tensor("cosT", [DH, T], bf16, kind="ExternalInput").ap()
    sinmT = nc.dram_tensor("sinmT", [DH, T], bf16, kind="ExternalInput").ap()
    out_rows = nc.dram_tensor("out_rows", [TS_C, D], f32,
                              kind="ExternalOutput").ap()

    Act = mybir.ActivationFunctionType
    Alu = mybir.AluOpType
    inv_sqrt_dh = 1.0 / math.sqrt(DH)
    MAGIC = 0x5F3759DF
    groups = [list(range(n_cores))]

    with tile.TileContext(nc) as tc, \
            tc.tile_pool(name="persist", bufs=1) as persist:
        # ---------------- long-lived tensors ----------------
        Q_sb = persist.tile([P, NH, T], bf16, tag="Q_sb")
        K_sb = persist.tile([P, NH, T], bf16, tag="K_sb")
        V_sb = persist.tile([P, NTS, DL], bf16, tag="V_sb")
        rcol_sb = persist.tile([P, NTS], f32, tag="rcol_sb")
        rrow_sb = persist.tile([1, T], bf16, tag="rrow_sb")
        ones_bf = persist.tile([P, 1], bf16, tag="ones_bf")
        ones_row = persist.tile([1, P], bf16, tag="ones_row")
        mask_sb = persist.tile([P, P], f32, tag="mask_sb")
        warm_sb = persist.tile([P, 128], bf16, tag="warm_sb")

        nc.vector.memset(ones_bf[:], 1.0)
        nc.vector.memset(ones_row[:], 1.0)
        nc.vector.memset(warm_sb[:], 0.0)

        with tc.tile_pool(name="dram", bufs=1, space="DRAM") as dram_pool:
            a2a_in = [dram_pool.tile([n_cores, DH, TS_C], bf16, tag=f"a2ai{h}",
                                     name=f"a2a_in{h}") for h in range(NH)]
            a2a_out = [dram_pool.tile([n_cores, DH, TS_C], bf16,
                                      tag=f"a2ao{h}", name=f"a2a_out{h}")
                       for h in range(NH)]
            dummy_i = dram_pool.tile([n_cores, 128], bf16, tag="dmi",
                                     name="dummy_i")
            dummy_o = dram_pool.tile([n_cores, 128], bf16, tag="dmo",
                                     name="dummy_o")

            # gpsimd queue carries ONLY the mask setup + collective triggers:
            # a collective trigger blocks the gpsimd queue until the
            # collective completes, so nothing latency-critical may follow.
            nc.gpsimd.memset(mask_sb[:], 0.0)
            # keep (0) where j <= i within the diagonal 128x128 block:
            # j = base + p, i = base + f  ->  keep f - p >= 0
            nc.gpsimd.affine_select(
                out=mask_sb[:], in_=mask_sb[:],
                pattern=[[1, P]], channel_multiplier=-1, base=0,
                compare_op=Alu.is_ge, fill=NEG)
            # warm the collective path (absorbs the cross-core entry barrier
            # + first-op setup while the DMA-bound prologue runs)
            nc.sync.dma_start(dummy_i[:], warm_sb[:n_cores, :])
            nc.gpsimd.collective_compute(
                "AllToAll", Alu.bypass, replica_groups=groups,
                ins=[dummy_i[:].opt()], outs=[dummy_o[:].opt()])

            # PE warmup: back-to-back dummy matmuls so the HAM clock gate
            # opens before the real work arrives
            with tc.tile_pool(name="warm_ps", bufs=1, space="PSUM") as wmps:
                wps = wmps.tile([P, 128], f32, tag="wm")
                for _ in range(32):
                    nc.tensor.matmul(wps[:], warm_sb[:], warm_sb[:],
                                     start=True, stop=True)

            import contextlib
            with contextlib.ExitStack() as stk_wo:
                wopool = stk_wo.enter_context(tc.tile_pool(name="wo", bufs=6))
                stk0 = stk_wo.enter_context(contextlib.ExitStack())
                _p = lambda *a, **kw: stk0.enter_context(tc.tile_pool(*a, **kw))
                wpool = _p(name="wqkv", bufs=1)
                cspool = _p(name="cs_raw", bufs=1)
                xpool = _p(name="xk", bufs=1)
                xrpool = _p(name="xrow", bufs=3)
                sqpool = _p(name="sqs", bufs=1)
                rpool = _p(name="rtmp", bufs=1)
                ropool = _p(name="rope", bufs=2)
                ptpool = _p(name="ptp", bufs=3)
                finpool = _p(name="fin", bufs=2)
                pps = _p(name="proj_ps", bufs=3, space="PSUM")
                stps = _p(name="st_ps", bufs=2, space="PSUM")
                avps = _p(name="av_ps", bufs=1, space="PSUM")
                rowps = _p(name="row_ps", bufs=1, space="PSUM")
                bcps = _p(name="bc_ps", bufs=1, space="PSUM")
                # ---- initial loads (sync queue, priority order) ----
                wq_sb = wpool.tile([P, KC, DL], bf16, tag="wq")
                wk_sb = wpool.tile([P, KC, DL], bf16, tag="wk")
                wv_sb = wpool.tile([P, KC, DL], bf16, tag="wv")
                xk = [xpool.tile([P, T], bf16, tag=f"xk{kc}", name=f"xk{kc}")
                      for kc in range(KC)]
                nc.sync.dma_start(wq_sb[:], wq_t.rearrange("p (kc j) -> p kc j", j=DL))
                for kc in range(KC):
                    nc.sync.dma_start(xk[kc][:], xT[P * kc:P * (kc + 1), :])
                nc.sync.dma_start(wk_sb[:], wk_t.rearrange("p (kc j) -> p kc j", j=DL))
                nc.sync.dma_start(wv_sb[:], wv_t.rearrange("p (kc j) -> p kc j", j=DL))
                cos_bf = cspool.tile([P, T], bf16, tag="cosb")
                sinm_bf = cspool.tile([P, T], bf16, tag="sinb")
                nc.scalar.dma_start(cos_bf[:], cosT)
                nc.scalar.dma_start(sinm_bf[:], sinmT)
                cos_r = cspool.tile([P, T], f32, tag="cos")
                sinm_r = cspool.tile([P, T], f32, tag="sin")

                # ---- RMSNorm row scales r = rsqrt(mean(x^2)+eps), 2 halves
                rs_raw = rpool.tile([P, NTS], f32, tag="rs_raw")
                rcol_bf = rpool.tile([P, NTS], bf16, tag="rcol_bf")
                mcol = rpool.tile([P, NTS], f32, tag="mcol")
                ri = rpool.tile([P, NTS], i32, tag="ri")
                tn = rpool.tile([P, NTS], f32, tag="tn")
                HC = NTS // 2  # chunks per half

                def r_half(half):
                    hsl = slice(HC * half, HC * (half + 1))
                    for ch in range(HC * half, HC * (half + 1)):
                        xr = xrpool.tile([P, D], bf16, tag="xr",
                                         name=f"xr{ch}")
                        nc.sync.dma_start(xr[:], x_rows[P * ch:P * (ch + 1), :])
                        scr = sqpool.tile([P, D], bf16, tag="scr")
                        nc.scalar.activation(scr[:], xr[:], Act.Square,
                                             accum_out=rs_raw[:, ch:ch + 1])
                    # r = rsqrt(sum/D + eps): bit-trick seed + 2 Newton
                    nc.vector.tensor_scalar(mcol[:, hsl], rs_raw[:, hsl],
                                            1.0 / D, EPS, Alu.mult, Alu.add)
                    nc.vector.tensor_scalar(ri[:, hsl],
                                            mcol[:, hsl].bitcast(i32), 1,
                                            None, Alu.arith_shift_right)
                    nc.vector.tensor_scalar(ri[:, hsl], ri[:, hsl], -1, MAGIC,
                                            Alu.mult, Alu.add)
                    rrv = ri[:, hsl].bitcast(f32)
                    for _ in range(2):
                        nc.vector.tensor_tensor(tn[:, hsl], rrv, rrv, Alu.mult)
                        nc.vector.tensor_tensor(tn[:, hsl], tn[:, hsl],
                                                mcol[:, hsl], Alu.mult)
                        nc.vector.tensor_scalar(tn[:, hsl], tn[:, hsl], -0.5,
                                                1.5, Alu.mult, Alu.add)
                        nc.vector.tensor_tensor(rrv, rrv, tn[:, hsl], Alu.mult)
                    nc.vector.tensor_copy(rcol_sb[:, hsl], rrv)
                    nc.vector.tensor_copy(rcol_bf[:, hsl], rrv)
                    for ch in range(HC * half, HC * (half + 1)):
                        nc.sync.dma_start(rrow_sb[0:1, P * ch:P * (ch + 1)],
                                          rcol_bf[:, ch:ch + 1])
                    # broadcast r across partitions on the PE (ones column x
                    # r row) and fold into the RoPE tables
                    for s in range(2 * half, 2 * half + 2):
                        tsl = slice(512 * s, 512 * (s + 1))
                        rps = bcps.tile([P, 512], f32, tag="bc",
                                        name=f"rbc{s}")
                        nc.tensor.matmul(rps[:], ones_row[:],
                                         rrow_sb[0:1, tsl],
                                         start=True, stop=True)
                        nc.vector.tensor_tensor(cos_r[:, tsl], cos_bf[:, tsl],
                                                rps[:], Alu.mult)
                        nc.vector.tensor_tensor(sinm_r[:, tsl],
                                                sinm_bf[:, tsl],
                                                rps[:], Alu.mult)

                r_half(0)
                r_half(1)

                # Wo weight chunks: stream during attention, spread queues
                wo_view = wo_t.rearrange("p (kc rg j) -> p kc rg j",
                                         kc=KC, rg=RG)
                wo_chunks = [None] * KC
                wo_order = [2 * k for k in range(KC // 2)] + \
                           [2 * k + 1 for k in range(KC // 2)]
                _wo_engs = [nc.sync, nc.scalar]

                def load_woc(pos):
                    kc = wo_order[pos]
                    woc = wopool.tile([P, RG, 512], bf16, tag="woc",
                                      name=f"woc{kc}")
                    _wo_engs[pos % 2].dma_start(woc[:], wo_view[:, kc, :, :])
                    wo_chunks[kc] = woc

                # ---- per-block QKV + attention, software-pipelined ----
                def qkv_block(B):
                    tb = slice(512 * B, 512 * B + 512)
                    for qk, (w_sb, dst) in enumerate(((wq_sb, Q_sb),
                                                      (wk_sb, K_sb))):
                        for h in range(NH):
                            hs = slice(DH * h, DH * (h + 1))
                            qp = pps.tile([P, 512], f32, tag="proj",
                                          name=f"p{B}_{qk}_{h}")
                            for kc in range(KC):
                                nc.tensor.matmul(qp[:], w_sb[:, kc, hs],
                                                 xk[kc][:, tb],
                                                 start=(kc == 0),
                                                 stop=(kc == KC - 1))
                            # RoPE on DVE: dst = qp*cos_r + swap64(qp*sinm_r)
                            tmp = ropool.tile([P, 512], bf16, tag="tmp")
                            tmp2 = ropool.tile([P, 512], bf16, tag="tmp2")
                            nc.vector.tensor_tensor(tmp[:], qp[:],
                                                    sinm_r[:, tb], Alu.mult)
                            nc.sync.dma_start(tmp2[0:64, :], tmp[64:128, :])
                            nc.sync.dma_start(tmp2[64:128, :], tmp[0:64, :])
                            nc.vector.tensor_tensor(dst[:, h, tb], qp[:],
                                                    cos_r[:, tb], Alu.mult)
                            nc.vector.tensor_tensor(dst[:, h, tb],
                                                    dst[:, h, tb], tmp2[:],
                                                    Alu.add)
                    for ts in range(4):
                        i = 4 * B + ts
                        tsl = slice(512 * B + P * ts, 512 * B + P * (ts + 1))
                        vp = pps.tile([P, 512], f32, tag="proj",
                                      name=f"pv{B}_{ts}")
                        for kc in range(KC):
                            nc.tensor.matmul(vp[:, :DL], xk[kc][:, tsl],
                                             wv_sb[:, kc, :], start=(kc == 0),
                                             stop=(kc == KC - 1))
                        nc.vector.tensor_scalar_mul(V_sb[:, i, :], vp[:, :DL],
                                                    rcol_sb[:, i:i + 1])

                def att_block(B):
                    for h in range(NH):
                        hs = slice(DH * h, DH * (h + 1))
                        av = avps.tile([P, 512], f32, tag="av",
                                       name=f"av{B}_{h}")
                        ssum = rowps.tile([1, 512], f32, tag="row",
                                          name=f"ss{B}_{h}")
                        Jmax = 4 * B + 3
                        for J in range(Jmax + 1):
                            r = J - 4 * B  # >=0 on the diagonal 512-block
                            lo = max(0, 128 * r)  # live i-range start
                            isl = slice(512 * B + lo, 512 * B + 512)
                            st = stps.tile([P, 512], f32, tag="st",
                                           name=f"st{B}_{h}_{J}")
                            nc.tensor.matmul(st[:, lo:],
                                             K_sb[:, h, P * J:P * (J + 1)],
                                             Q_sb[:, h, isl],
                                             start=True, stop=True)
                            if r >= 0:
                                nc.vector.tensor_tensor(
                                    st[:, lo:lo + P], st[:, lo:lo + P],
                                    mask_sb[:], Alu.add)
                            pt = ptpool.tile([P, 512], bf16, tag="pt")
                            nc.scalar.activation(pt[:, lo:], st[:, lo:],
                                                 Act.Exp, scale=inv_sqrt_dh)
                            nc.tensor.matmul(av[:, lo:], V_sb[:, J, hs],
                                             pt[:, lo:], start=(J == 0),
                                             stop=(J == Jmax),
                                             skip_group_check=True)
                            nc.tensor.matmul(ssum[:, lo:], ones_bf[:],
                                             pt[:, lo:], start=(J == 0),
                                             stop=(J == Jmax),
                                             skip_group_check=True)
                        rinv = finpool.tile([1, 512], f32, tag="rinv")
                        nc.vector.reciprocal_approx_fast(rinv[:], ssum[:])
                        rinv_bf = finpool.tile([1, 512], bf16, tag="rinvb")
                        nc.vector.tensor_copy(rinv_bf[:], rinv[:])
                        rb = bcps.tile([P, 512], f32, tag="bc",
                                       name=f"rb{B}_{h}")
                        nc.tensor.matmul(rb[:], ones_row[:], rinv_bf[:],
                                         start=True, stop=True)
                        rb_sb = finpool.tile([P, 512], f32, tag="rbsb")
                        nc.scalar.activation(rb_sb[:], rb[:], Act.Copy)
                        att = finpool.tile([P, 512], bf16, tag="att")
                        nc.vector.tensor_tensor(att[:], av[:], rb_sb[:],
                                                Alu.mult)
                        # store to the AllToAll source: dest core 2B+c' gets
                        # i-window att[:, 256c' : 256c'+256]
                        nc.sync.dma_start(
                            a2a_in[h][2 * B:2 * B + 2, :, :]
                            .rearrange("c p i -> p c i"),
                            att[:])

                qkv_block(0)
                load_woc(0)
                load_woc(1)
                qkv_block(1)
                att_block(0)
                for pos in range(2, 6):
                    load_woc(pos)
                qkv_block(2)
                att_block(1)
                for pos in range(6, 11):
                    load_woc(pos)
                qkv_block(3)
                att_block(2)
                for pos in range(11, KC):
                    load_woc(pos)
                att_block(3)

                for h in range(NH):
                    nc.gpsimd.collective_compute(
                        "AllToAll", Alu.bypass, replica_groups=groups,
                        ins=[a2a_in[h][:].opt()], outs=[a2a_out[h][:].opt()])

                # ============== output projection (sequence-sharded) =====
                stk0.close()  # free attention SBUF + PSUM pools
                with contextlib.ExitStack() as stk:
                    xcpool = stk.enter_context(tc.tile_pool(name="xc", bufs=2))
                    opool = stk.enter_context(tc.tile_pool(name="osb", bufs=2))
                    ops = stk.enter_context(
                        tc.tile_pool(name="o_ps", bufs=8, space="PSUM"))
                    # att_all reuses Q_sb's SBUF (dead after the last scores)
                    att_all = Q_sb[:].rearrange("p a (c i) -> p (a c) i",
                                                i=TS_C)
                    for h in range(NH):
                        avf = a2a_out[h][:].rearrange("c d i -> (c d) i")
                        for cc in range(n_cores):
                            kc = 2 * cc + h
                            nc.sync.dma_start(att_all[:, kc, :],
                                              avf[P * cc:P * (cc + 1), :])
                    outp = [ops.tile([P, 512], f32, tag="om", name=f"om{t}")
                            for t in range(8)]
                    for pos, kc in enumerate(wo_order):
                        for isub in range(2):
                            for rg in range(RG):
                                nc.tensor.matmul(
                                    outp[4 * isub + rg][:],
                                    att_all[:, kc, P * isub:P * (isub + 1)],
                                    wo_chunks[kc][:, rg, :],
                                    start=(pos == 0), stop=(pos == KC - 1),
                                    skip_group_check=True)
                    for isub in range(2):
                        for rg in range(RG):
                            rsl = slice(512 * rg, 512 * (rg + 1))
                            xct = xcpool.tile([P, 512], f32, tag="xct")
                            nc.scalar.dma_start(
                                xct[:], x_seq[P * isub:P * (isub + 1), rsl])
                            osb = opool.tile([P, 512], f32, tag="osb")
                            nc.vector.tensor_tensor(osb[:],
                                                    outp[4 * isub + rg][:],
                                                    xct[:], Alu.add)
                            nc.sync.dma_start(
                                out_rows[P * isub:P * (isub + 1), rsl],
                                osb[:])

    nc.compile()
    return nc


# --------------------------------------------------------------------------
# host-side prep / entry point
# --------------------------------------------------------------------------
def prepare_inputs(x, cos, sin, ln_w, Wq, Wk, Wv, Wo, n_cores, heads_per_core):
    import ml_dtypes
    bf16 = ml_dtypes.bfloat16
    DH = 128
    DL = heads_per_core * DH
    x = np.ascontiguousarray(np.asarray(x, dtype=np.float32))
    T, D = x.shape
    KC = D // DH
    TS_C = T // n_cores
    cosT = np.ascontiguousarray(np.asarray(cos, np.float32).T.astype(bf16))
    sinmT = np.asarray(sin, np.float32).T.copy()
    sinmT[64:, :] *= -1.0  # sign fold for the rotate-half swap trick
    sinmT = np.ascontiguousarray(sinmT.astype(bf16))
    lnw = np.asarray(ln_w, np.float32)
    xT = np.ascontiguousarray(x.T.astype(bf16))
    x_rows = np.ascontiguousarray(x.astype(bf16))

    def pretile_qkv(W, cols):
        # rows j of W (out dims), ln_w folded; SBUF layout [P, KC*DL]
        arr = (np.asarray(W, np.float32)[cols, :] * lnw[None, :]).T  # (D, DL)
        return np.ascontiguousarray(
            arr.reshape(KC, DH, DL).transpose(1, 0, 2).reshape(DH, KC * DL)
            .astype(bf16))

    # Wo full, pretiled: element (p, kc, rg, j) = Wo.T[128kc+p, 512rg+j]
    woT = np.asarray(Wo, np.float32).T  # (D, D) = (d_in, d_out)
    wo_t = np.ascontiguousarray(
        woT.reshape(KC, DH, 4, 512).transpose(1, 0, 2, 3)
        .reshape(DH, KC * 4 * 512).astype(bf16))

    in_maps = []
    for c in range(n_cores):
        cols = slice(c * DL, (c + 1) * DL)
        rows = slice(c * TS_C, (c + 1) * TS_C)
        in_maps.append({
            "xT": xT,
            "x_rows": x_rows,
            "x_seq": np.ascontiguousarray(x[rows, :]),
            "wq_t": pretile_qkv(Wq, cols),
            "wk_t": pretile_qkv(Wk, cols),
            "wv_t": pretile_qkv(Wv, cols),
            "wo_t": wo_t,
            "cosT": cosT,
            "sinmT": sinmT,
        })
    return in_maps


_NC_CACHE = {}


def kernel(x, cos, sin, attention_mask, ln_w, Wq, Wk, Wv, Wo,
           _trace=False, _trace_cores=None):
    from concourse.bass_utils import run_bass_kernel_spmd

    cfg = CFG_FULL
    key = tuple(sorted(cfg.items()))
    if key not in _NC_CACHE:
        _NC_CACHE[key] = build_nc(**cfg)
    nc = _NC_CACHE[key]
    n_cores = cfg["n_cores"]
    in_maps = prepare_inputs(x, cos, sin, ln_w, Wq, Wk, Wv, Wo,
                             n_cores, cfg["heads_per_core"])
    res = run_bass_kernel_spmd(nc, in_maps, core_ids=list(range(n_cores)),
                               trace=_trace, trace_cores=_trace_cores)
    out = np.concatenate(
        [res.results[c]["out_rows"] for c in range(n_cores)], axis=0)
    kernel.last_result = res
    return out


# revision 28
# speedup vs baseline: 1.2149x; 1.0991x over previous
"""Trainium2 Bass kernel for a dense-transformer attention block.

Reference semantics (T=2048, D=2048, 16 heads, d_h=128):
    h = RMSNorm(x) * ln_w
    q,k,v = h @ W{q,k,v}.T  -> (n_h, T, d_h);  RoPE(q, k)
    att = softmax(causal(q k^T / sqrt(d_h))) @ v
    out = x + att @ Wo.T          (attention_mask is all-ones per spec)

Distribution: head-parallel over 8 cores (2 heads/core) for QKV+attention;
the output is SEQUENCE-sharded: one AllToAll redistributes att^T so each
core holds all 2048 att rows for its own 256 output timesteps, then each
core runs the FULL Wo against its timestep slice (~8x fewer collective
bytes than all-gathering att).

Engine plan per core:
  r-chain  RMSNorm row scales r[t] from a row-major x copy; squares split
           between ScalarE (Act Square + accum_out) and DVE
           (tensor_tensor_reduce), x_rows loads paced on the Act queue so
           they never block the xT stream; bit-trick rsqrt Newton on DVE in
           [128,8] column layout (two halves); r broadcast across
           partitions via a ones-column PE matmul, folded into bf16-loaded
           RoPE tables (f32 result).
  QKV      per 512-block B: bf16 matmuls (ln_w folded into weights on
           host); Q/K PSUM drained raw to SBUF so the PE never waits on r;
           RoPE applied in-place on DVE with a sign-folded sin table and a
           partition-half-swap SBUF DMA.
  att      causal attention, scores transposed (S^T[j,i]); i-range sliced
           to skip fully-masked diagonal work; softmax row-sums on the PE
           via a ones-vector matmul; exp on ScalarE; 1/sum broadcast via
           ones-column PE matmul.  gpsimd carries ONLY mask setup and
           collective triggers (a trigger blocks the gpsimd queue).
  Wo       out[i,:] = att_all^T @ Wo, weights streamed during attention on
           spread DMA queues; residual added from a bf16 x slice preloaded
           into dead K_sb space; per-tile drains inline with the last
           accumulation pass.
Host assembles out = concat(out_rows, axis=0).
"""

import contextlib
import math

import numpy as np

EPS = 1e-5
NEG = -1.0e30

CFG_FULL = dict(T=2048, D=2048, n_cores=8, heads_per_core=2)


# --------------------------------------------------------------------------
# device program
# --------------------------------------------------------------------------
def build_nc(T, D, n_cores, heads_per_core):
    import concourse.mybir as mybir
    import concourse.tile as tile
    from concourse import bacc

    DH = 128                      # head dim (hard-wired into layout)
    P = 128                       # partitions
    NH = heads_per_core
    DL = NH * DH                  # local width (q/k/v columns per core)
    KC = D // P                   # 128-contraction chunks over d_model
    NTS = T // P                  # 128-wide t subtiles
    TS_C = T // n_cores           # output timesteps per core (256)
    RG = 4                        # Wo column groups of 512
    f32 = mybir.dt.float32
    bf16 = mybir.dt.bfloat16
    i32 = mybir.dt.int32

    nc = bacc.Bacc("TRN2", target_bir_lowering=False, debug=False,
                   num_devices=n_cores)

    # ---- I/O ----
    xT = nc.dram_tensor("xT", [D, T], bf16, kind="ExternalInput").ap()
    x_rows = nc.dram_tensor("x_rows", [T, D], bf16, kind="ExternalInput").ap()
    x_seq = nc.dram_tensor("x_seq", [TS_C, D], bf16, kind="ExternalInput").ap()
    wq_t = nc.dram_tensor("wq_t", [P, KC * DL], bf16, kind="ExternalInput").ap()
    wk_t = nc.dram_tensor("wk_t", [P, KC * DL], bf16, kind="ExternalInput").ap()
    wv_t = nc.dram_tensor("wv_t", [P, KC * DL], bf16, kind="ExternalInput").ap()
    wo_t = nc.dram_tensor("wo_t", [P, KC * RG * 512], bf16,
                          kind="ExternalInput").ap()
    cosT = nc.dram_tensor("cosT", [DH, T], bf16, kind="ExternalInput").ap()
    sinmT = nc.dram_tensor("sinmT", [DH, T], bf16, kind="ExternalInput").ap()
    out_rows = nc.dram_tensor("out_rows", [TS_C, D], f32,
                              kind="ExternalOutput").ap()

    Act = mybir.ActivationFunctionType
    Alu = mybir.AluOpType
    inv_sqrt_dh = 1.0 / math.sqrt(DH)
    MAGIC = 0x5F3759DF
    groups = [list(range(n_cores))]

    with tile.TileContext(nc) as tc, \
            tc.tile_pool(name="persist", bufs=1) as persist:
        # ---------------- long-lived tensors ----------------
        Q_sb = persist.tile([P, NH, T], bf16, tag="Q_sb")
        K_sb = persist.tile([P, NH, T], bf16, tag="K_sb")
        V_sb = persist.tile([P, NTS, DL], bf16, tag="V_sb")
        rcol_sb = persist.tile([P, NTS], f32, tag="rcol_sb")
        rrow_sb = persist.tile([1, T], bf16, tag="rrow_sb")
        ones_bf = persist.tile([P, 1], bf16, tag="ones_bf")
        ones_row = persist.tile([1, P], bf16, tag="ones_row")
        mask_sb = persist.tile([P, P], f32, tag="mask_sb")
        warm_sb = persist.tile([P, 128], bf16, tag="warm_sb")

        nc.vector.memset(ones_bf[:], 1.0)
        nc.vector.memset(ones_row[:], 1.0)
        nc.vector.memset(warm_sb[:], 0.0)

        with tc.tile_pool(name="dram", bufs=1, space="DRAM") as dram_pool:
            a2a_in = dram_pool.tile([n_cores, DL, TS_C], bf16, tag="a2ai",
                                    name="a2a_in")
            a2a_out = dram_pool.tile([n_cores, DL, TS_C], bf16, tag="a2ao",
                                     name="a2a_out")
            dummy_i = dram_pool.tile([n_cores, 128], bf16, tag="dmi",
                                     name="dummy_i")
            dummy_o = dram_pool.tile([n_cores, 128], bf16, tag="dmo",
                                     name="dummy_o")

            # gpsimd queue carries ONLY the mask setup + collective
            # triggers: a collective trigger blocks the gpsimd queue until
            # the collective completes.
            nc.gpsimd.memset(mask_sb[:], 0.0)
            # keep (0) where j <= i within the diagonal 128x128 block:
            # j = base + p, i = base + f  ->  keep f - p >= 0
            nc.gpsimd.affine_select(
                out=mask_sb[:], in_=mask_sb[:],
                pattern=[[1, P]], channel_multiplier=-1, base=0,
                compare_op=Alu.is_ge, fill=NEG)
            # warm the collective path (absorbs the cross-core entry
            # barrier + first-op setup while the DMA-bound prologue runs)
            nc.sync.dma_start(dummy_i[:], warm_sb[:n_cores, :])
            nc.gpsimd.collective_compute(
                "AllToAll", Alu.bypass, replica_groups=groups,
                ins=[dummy_i[:].opt()], outs=[dummy_o[:].opt()])

            # PE warmup: back-to-back dummy matmuls so the HAM clock gate
            # opens before the real work arrives
            with tc.tile_pool(name="warm_ps", bufs=1, space="PSUM") as wmps:
                wps = wmps.tile([P, 128], f32, tag="wm")
                for _ in range(40):
                    nc.tensor.matmul(wps[:], warm_sb[:], warm_sb[:],
                                     start=True, stop=True)

            with contextlib.ExitStack() as stk_wo:
                wopool = stk_wo.enter_context(tc.tile_pool(name="wo", bufs=6))
                stk0 = stk_wo.enter_context(contextlib.ExitStack())
                _p = lambda *a, **kw: stk0.enter_context(tc.tile_pool(*a, **kw))
                wpool = _p(name="wqkv", bufs=1)
                cspool = _p(name="cs_raw", bufs=1)
                xpool = _p(name="xk", bufs=1)
                xrpool = _p(name="xrow", bufs=3)
                sqpool = _p(name="sqs", bufs=1)
                rpool = _p(name="rtmp", bufs=1)
                ropool = _p(name="rope", bufs=2)
                ptpool = _p(name="ptp", bufs=3)
                finpool = _p(name="fin", bufs=2)
                pps = _p(name="proj_ps", bufs=3, space="PSUM")
                stps = _p(name="st_ps", bufs=2, space="PSUM")
                avps = _p(name="av_ps", bufs=1, space="PSUM")
                rowps = _p(name="row_ps", bufs=1, space="PSUM")
                bcps = _p(name="bc_ps", bufs=1, space="PSUM")

                # ---- initial loads: xT stream on sync; x_rows paced on the
                # Act queue (so squares never block xT); tables on Act queue
                wq_sb = wpool.tile([P, KC, DL], bf16, tag="wq")
                wk_sb = wpool.tile([P, KC, DL], bf16, tag="wk")
                wv_sb = wpool.tile([P, KC, DL], bf16, tag="wv")
                xk = [xpool.tile([P, T], bf16, tag=f"xk{kc}", name=f"xk{kc}")
                      for kc in range(KC)]
                cos_bf = cspool.tile([P, T], bf16, tag="cosb")
                sinm_bf = cspool.tile([P, T], bf16, tag="sinb")
                nc.scalar.dma_start(cos_bf[:], cosT)
                nc.scalar.dma_start(sinm_bf[:], sinmT)
                nc.sync.dma_start(wq_sb[:], wq_t.rearrange("p (kc j) -> p kc j", j=DL))
                for kc in range(KC):
                    nc.sync.dma_start(xk[kc][:], xT[P * kc:P * (kc + 1), :])
                nc.sync.dma_start(wk_sb[:], wk_t.rearrange("p (kc j) -> p kc j", j=DL))
                nc.sync.dma_start(wv_sb[:], wv_t.rearrange("p (kc j) -> p kc j", j=DL))
                cos_r = cspool.tile([P, T], f32, tag="cos")
                sinm_r = cspool.tile([P, T], f32, tag="sin")

                # ---- RMSNorm row scales r = rsqrt(mean(x^2)+eps) ----
                rs_raw = rpool.tile([P, NTS], f32, tag="rs_raw")
                rcol_bf = rpool.tile([P, NTS], bf16, tag="rcol_bf")
                mcol = rpool.tile([P, NTS], f32, tag="mcol")
                ri = rpool.tile([P, NTS], i32, tag="ri")
                tn = rpool.tile([P, NTS], f32, tag="tn")
                HC = NTS // 2  # chunks per half

                def r_sums(half):
                    hsl = slice(HC * half, HC * (half + 1))
                    for ch in range(HC * half, HC * (half + 1)):
                        xr = xrpool.tile([P, D], bf16, tag="xr",
                                         name=f"xr{ch}")
                        nc.scalar.dma_start(xr[:],
                                            x_rows[P * ch:P * (ch + 1), :])
                        acc = rs_raw[:, ch:ch + 1]
                        scr = sqpool.tile([P, D], bf16, tag="scrA")
                        nc.scalar.activation(scr[:], xr[:], Act.Square,
                                             accum_out=acc)
                    # r = rsqrt(sum/D + eps): bit-trick seed + 2 Newton
                    nc.vector.tensor_scalar(mcol[:, hsl], rs_raw[:, hsl],
                                            1.0 / D, EPS, Alu.mult, Alu.add)
                    nc.vector.tensor_scalar(ri[:, hsl],
                                            mcol[:, hsl].bitcast(i32), 1,
                                            None, Alu.arith_shift_right)
                    nc.vector.tensor_scalar(ri[:, hsl], ri[:, hsl], -1, MAGIC,
                                            Alu.mult, Alu.add)
                    rrv = ri[:, hsl].bitcast(f32)
                    for _ in range(2):
                        nc.vector.tensor_tensor(tn[:, hsl], rrv, rrv, Alu.mult)
                        nc.vector.tensor_tensor(tn[:, hsl], tn[:, hsl],
                                                mcol[:, hsl], Alu.mult)
                        nc.vector.tensor_scalar(tn[:, hsl], tn[:, hsl], -0.5,
                                                1.5, Alu.mult, Alu.add)
                        nc.vector.tensor_tensor(rrv, rrv, tn[:, hsl], Alu.mult)
                    nc.vector.tensor_copy(rcol_sb[:, hsl], rrv)
                    nc.vector.tensor_copy(rcol_bf[:, hsl], rrv)
                    for ch in range(HC * half, HC * (half + 1)):
                        nc.sync.dma_start(rrow_sb[0:1, P * ch:P * (ch + 1)],
                                          rcol_bf[:, ch:ch + 1])

                def r_tables(half):
                    # broadcast r across partitions on the PE (ones column x
                    # r row) and fold into the RoPE tables
                    for s in range(2 * half, 2 * half + 2):
                        tsl = slice(512 * s, 512 * (s + 1))
                        rps = bcps.tile([P, 512], f32, tag="bc",
                                        name=f"rbc{s}")
                        nc.tensor.matmul(rps[:], ones_row[:],
                                         rrow_sb[0:1, tsl],
                                         start=True, stop=True)
                        nc.vector.tensor_tensor(cos_r[:, tsl], cos_bf[:, tsl],
                                                rps[:], Alu.mult)
                        nc.vector.tensor_tensor(sinm_r[:, tsl],
                                                sinm_bf[:, tsl],
                                                rps[:], Alu.mult)

                # Wo weight chunks: stream during attention, spread queues
                wo_view = wo_t.rearrange("p (kc rg j) -> p kc rg j",
                                         kc=KC, rg=RG)
                wo_chunks = [None] * KC
                _wo_engs = [nc.sync, nc.scalar]

                def load_woc(kc):
                    woc = wopool.tile([P, RG, 512], bf16, tag="woc",
                                      name=f"woc{kc}")
                    _wo_engs[kc % 2].dma_start(woc[:], wo_view[:, kc, :, :])
                    wo_chunks[kc] = woc

                # ---- per-block QKV matmuls (PE decoupled from r) ----
                def qkv_mms(B):
                    tb = slice(512 * B, 512 * B + 512)
                    for qk, (w_sb, dst) in enumerate(((wq_sb, Q_sb),
                                                      (wk_sb, K_sb))):
                        for h in range(NH):
                            hs = slice(DH * h, DH * (h + 1))
                            qp = pps.tile([P, 512], f32, tag="proj",
                                          name=f"p{B}_{qk}_{h}")
                            for kc in range(KC):
                                nc.tensor.matmul(qp[:], w_sb[:, kc, hs],
                                                 xk[kc][:, tb],
                                                 start=(kc == 0),
                                                 stop=(kc == KC - 1))
                            # raw drain; RoPE comes later, in place
                            nc.vector.tensor_copy(dst[:, h, tb], qp[:])
                    for ts in range(4):
                        i = 4 * B + ts
                        tsl = slice(512 * B + P * ts, 512 * B + P * (ts + 1))
                        vp = pps.tile([P, 512], f32, tag="proj",
                                      name=f"pv{B}_{ts}")
                        for kc in range(KC):
                            nc.tensor.matmul(vp[:, :DL], xk[kc][:, tsl],
                                             wv_sb[:, kc, :], start=(kc == 0),
                                             stop=(kc == KC - 1))
                        nc.vector.tensor_scalar_mul(V_sb[:, i, :], vp[:, :DL],
                                                    rcol_sb[:, i:i + 1])

                def rope(B):
                    tb = slice(512 * B, 512 * B + 512)
                    for dst in (Q_sb, K_sb):
                        for h in range(NH):
                            # dst = dst*cos_r + swap64(dst*sinm_r), in place
                            tmp = ropool.tile([P, 512], bf16, tag="tmp")
                            tmp2 = ropool.tile([P, 512], bf16, tag="tmp2")
                            nc.vector.tensor_tensor(tmp[:], dst[:, h, tb],
                                                    sinm_r[:, tb], Alu.mult)
                            nc.sync.dma_start(tmp2[0:64, :], tmp[64:128, :])
                            nc.sync.dma_start(tmp2[64:128, :], tmp[0:64, :])
                            nc.vector.tensor_tensor(dst[:, h, tb],
                                                    dst[:, h, tb],
                                                    cos_r[:, tb], Alu.mult)
                            nc.vector.tensor_tensor(dst[:, h, tb],
                                                    dst[:, h, tb], tmp2[:],
                                                    Alu.add)

                def att_block(B):
                    for h in range(NH):
                        hs = slice(DH * h, DH * (h + 1))
                        av = avps.tile([P, 512], f32, tag="av",
                                       name=f"av{B}_{h}")
                        ssum = rowps.tile([1, 512], f32, tag="row",
                                          name=f"ss{B}_{h}")
                        Jmax = 4 * B + 3
                        for J in range(Jmax + 1):
                            r = J - 4 * B  # >=0 on the diagonal 512-block
                            lo = max(0, 128 * r)  # live i-range start
                            isl = slice(512 * B + lo, 512 * B + 512)
                            st = stps.tile([P, 512], f32, tag="st",
                                           name=f"st{B}_{h}_{J}")
                            nc.tensor.matmul(st[:, lo:],
                                             K_sb[:, h, P * J:P * (J + 1)],
                                             Q_sb[:, h, isl],
                                             start=True, stop=True)
                            if r >= 0:
                                nc.vector.tensor_tensor(
                                    st[:, lo:lo + P], st[:, lo:lo + P],
                                    mask_sb[:], Alu.add)
                            pt = ptpool.tile([P, 512], bf16, tag="pt")
                            nc.scalar.activation(pt[:, lo:], st[:, lo:],
                                                 Act.Exp, scale=inv_sqrt_dh)
                            nc.tensor.matmul(av[:, lo:], V_sb[:, J, hs],
                                             pt[:, lo:], start=(J == 0),
                                             stop=(J == Jmax),
                                             skip_group_check=True)
                            nc.tensor.matmul(ssum[:, lo:], ones_bf[:],
                                             pt[:, lo:], start=(J == 0),
                                             stop=(J == Jmax),
                                             skip_group_check=True)
                        rinv = finpool.tile([1, 512], f32, tag="rinv")
                        nc.vector.reciprocal_approx_fast(rinv[:], ssum[:])
                        rinv_bf = finpool.tile([1, 512], bf16, tag="rinvb")
                        nc.vector.tensor_copy(rinv_bf[:], rinv[:])
                        rb = bcps.tile([P, 512], f32, tag="bc",
                                       name=f"rb{B}_{h}")
                        nc.tensor.matmul(rb[:], ones_row[:], rinv_bf[:],
                                         start=True, stop=True)
                        rb_sb = finpool.tile([P, 512], f32, tag="rbsb")
                        nc.scalar.activation(rb_sb[:], rb[:], Act.Copy)
                        att = finpool.tile([P, 512], bf16, tag="att")
                        nc.vector.tensor_tensor(att[:], av[:], rb_sb[:],
                                                Alu.mult)
                        # store to the AllToAll source: dest core 2B+c' gets
                        # i-window att[:, 256c' : 256c'+256]
                        nc.sync.dma_start(
                            a2a_in[2 * B:2 * B + 2, DH * h:DH * (h + 1), :]
                            .rearrange("c p i -> p c i"),
                            att[:])

                r_sums(0)
                r_sums(1)
                qkv_mms(0)
                r_tables(0)
                r_tables(1)
                rope(0)
                qkv_mms(1)
                rope(1)
                load_woc(0)
                load_woc(1)
                att_block(0)
                qkv_mms(2)
                rope(2)
                for kc in range(2, 6):
                    load_woc(kc)
                att_block(1)
                qkv_mms(3)
                rope(3)
                for kc in range(6, 11):
                    load_woc(kc)
                att_block(2)
                for kc in range(11, KC):
                    load_woc(kc)
                att_block(3)

                # residual rows, bf16, into dead K_sb space
                xs = K_sb  # [P, 2, T] view: (p, isub, rcol)
                for isub in range(2):
                    nc.sync.dma_start(xs[:, isub, :],
                                      x_seq[P * isub:P * (isub + 1), :])

                nc.gpsimd.collective_compute(
                    "AllToAll", Alu.bypass, replica_groups=groups,
                    ins=[a2a_in[:].opt()], outs=[a2a_out[:].opt()])

                # ============== output projection (sequence-sharded) =====
                stk0.close()  # free attention SBUF + PSUM pools
                with contextlib.ExitStack() as stk:
                    opool = stk.enter_context(tc.tile_pool(name="osb", bufs=2))
                    ops = stk.enter_context(
                        tc.tile_pool(name="o_ps", bufs=8, space="PSUM"))
                    # att_all reuses Q_sb's SBUF (dead after last scores)
                    att_all = Q_sb[:].rearrange("p a (c i) -> p (a c) i",
                                                i=TS_C)
                    avf = a2a_out[:].rearrange("c d i -> (c d) i")
                    for cc in range(n_cores):
                        nc.sync.dma_start(
                            att_all[:, 2 * cc:2 * cc + 2, :],
                            avf[DL * cc:DL * (cc + 1), :]
                            .rearrange("(a p) i -> p a i", p=P))
                    outp = [ops.tile([P, 512], f32, tag="om", name=f"om{t}")
                            for t in range(8)]
                    for kc in range(KC):
                        for isub in range(2):
                            for rg in range(RG):
                                t = 4 * isub + rg
                                nc.tensor.matmul(
                                    outp[t][:],
                                    att_all[:, kc, P * isub:P * (isub + 1)],
                                    wo_chunks[kc][:, rg, :],
                                    start=(kc == 0), stop=(kc == KC - 1))
                                if kc == KC - 1:
                                    rsl = slice(512 * rg, 512 * (rg + 1))
                                    osb = opool.tile([P, 512], f32,
                                                     tag="osb",
                                                     name=f"osb{t}")
                                    nc.vector.tensor_tensor(
                                        osb[:], outp[t][:],
                                        xs[:, isub, rsl], Alu.add)
                                    nc.sync.dma_start(
                                        out_rows[P * isub:P * (isub + 1),
                                                 rsl], osb[:])

    nc.compile()
    return nc


# --------------------------------------------------------------------------
# host-side prep / entry point
# --------------------------------------------------------------------------
def prepare_inputs(x, cos, sin, ln_w, Wq, Wk, Wv, Wo, n_cores, heads_per_core):
    import ml_dtypes
    bf16 = ml_dtypes.bfloat16
    DH = 128
    DL = heads_per_core * DH
    x = np.ascontiguousarray(np.asarray(x, dtype=np.float32))
    T, D = x.shape
    KC = D // DH
    TS_C = T // n_cores
    cosT = np.ascontiguousarray(np.asarray(cos, np.float32).T.astype(bf16))
    sinmT = np.asarray(sin, np.float32).T.copy()
    sinmT[64:, :] *= -1.0  # sign fold for the rotate-half swap trick
    sinmT = np.ascontiguousarray(sinmT.astype(bf16))
    lnw = np.asarray(ln_w, np.float32)
    xT = np.ascontiguousarray(x.T.astype(bf16))
    x_rows = np.ascontiguousarray(x.astype(bf16))

    def pretile_qkv(W, cols):
        # rows j of W (out dims), ln_w folded; SBUF layout [P, KC*DL]
        arr = (np.asarray(W, np.float32)[cols, :] * lnw[None, :]).T  # (D, DL)
        return np.ascontiguousarray(
            arr.reshape(KC, DH, DL).transpose(1, 0, 2).reshape(DH, KC * DL)
            .astype(bf16))

    # Wo full, pretiled: element (p, kc, rg, j) = Wo.T[128kc+p, 512rg+j]
    woT = np.asarray(Wo, np.float32).T  # (D, D) = (d_in, d_out)
    wo_t = np.ascontiguousarray(
        woT.reshape(KC, DH, 4, 512).transpose(1, 0, 2, 3)
        .reshape(DH, KC * 4 * 512).astype(bf16))

    in_maps = []
    for c in range(n_cores):
        cols = slice(c * DL, (c + 1) * DL)
        rows = slice(c * TS_C, (c + 1) * TS_C)
        in_maps.append({
            "xT": xT,
            "x_rows": x_rows,
            "x_seq": x_rows[rows, :],
            "wq_t": pretile_qkv(Wq, cols),
            "wk_t": pretile_qkv(Wk, cols),
            "wv_t": pretile_qkv(Wv, cols),
            "wo_t": wo_t,
            "cosT": cosT,
            "sinmT": sinmT,
        })
    return in_maps


_NC_CACHE = {}


def kernel(x, cos, sin, attention_mask, ln_w, Wq, Wk, Wv, Wo,
           _trace=False, _trace_cores=None):
    from concourse.bass_utils import run_bass_kernel_spmd

    cfg = CFG_FULL
    key = tuple(sorted(cfg.items()))
    if key not in _NC_CACHE:
        _NC_CACHE[key] = build_nc(**cfg)
    nc = _NC_CACHE[key]
    n_cores = cfg["n_cores"]
    in_maps = prepare_inputs(x, cos, sin, ln_w, Wq, Wk, Wv, Wo,
                             n_cores, cfg["heads_per_core"])
    res = run_bass_kernel_spmd(nc, in_maps, core_ids=list(range(n_cores)),
                               trace=_trace, trace_cores=_trace_cores)
    out = np.concatenate(
        [res.results[c]["out_rows"] for c in range(n_cores)], axis=0)
    kernel.last_result = res
    return out
